# revision 34
# baseline (speedup 1.0000x reference)
"""Trainium2 Bass kernel for nn_CustomModel_4372276707887 (GCLSTM stack).

Mathematical structure: in the reference, every GCLSTM step runs with
H = C = 0, so each ChebConv acts on a zero matrix and contributes only its
bias; the forget gate multiplies C=0 and the second round of blocks is
discarded.  The whole model therefore reduces to a dense per-node chain

    I = sigmoid(x @ W_i + cb_i + b_i)
    T = tanh  (x @ W_c + cb_c + b_c)
    C = I * T
    O = sigmoid(x @ W_o + cb_o + wc_o * C + b_o)
    H = relu(O * tanh(C))                 (relu from the block)
    h = relu(H @ c1_w + c1_b)
    ... same gate block with g2 (32 -> 16) ...
    L = relu(h2 @ c2_w + c2_b)
    y = L @ lin_w + lin_b                 out = concat([y, y], axis=0)

which never touches edge_index / edge_weight.  The kernel shards the 50000
nodes across 8 NeuronCores (6250 each, padded to 6272) and evaluates the
chain in a transposed layout (features on partitions, nodes on the free
dim) with block-diagonal weight packing so every engine op runs with full
128-partition occupancy.  The wc_o * C terms are folded into the O-gate
matmul as an accumulating diagonal matmul on the PE.  All weights and
biases ship in a single [128, SLAB_COLS] slab (one DMA); node tiles load
with one strided DMA per half-quad and store with one DMA per quad.
"""

import sys

if "/opt/trn_rl_repo" not in sys.path:
    sys.path.insert(0, "/opt/trn_rl_repo")

import numpy as np

N_CORES = 8
N_NODES = 50000
F_IN = 64
NLOC = N_NODES // N_CORES      # 6250 nodes per core
QN = 512                       # nodes per chunk in a full group
TAILN = 32                     # chunk size of the tail mini-group (4 * 32 = 128)
NPAD = 12 * QN + 4 * TAILN     # 6272 padded nodes per core (octet + quad + tail)

# weight-slab column layout: name -> (col_offset, n_cols, n_rows)
_WSLAB = {
    "A_i": (0, 128, 128),
    "A_c": (128, 128, 128),
    "A_o": (256, 128, 128),
    "Dwc1": (384, 128, 128),
    "Ac1": (512, 64, 128),
    "G2_i": (576, 64, 128),
    "G2_c": (640, 64, 128),
    "G2_o": (704, 64, 128),
    "Dwc2": (768, 64, 128),
    "Ac2": (832, 128, 128),
    "Alin": (960, 2, 128),
}
WSLAB_COLS = 962
_BSLAB = {
    "ident": (9, 128, 128),
    "b1_i": (0, 1, 128),
    "b1_c": (1, 1, 128),
    "b1_o": (2, 1, 128),
    "bc1": (3, 1, 64),
    "b2_i": (4, 1, 128),
    "b2_c": (5, 1, 128),
    "b2_o": (6, 1, 128),
    "bc2": (7, 1, 128),
    "blin": (8, 1, 2),
}
BSLAB_COLS = 137

_PROGRAM_CACHE = {}


def _blkdiag(*ms):
    k = sum(m.shape[0] for m in ms)
    n = sum(m.shape[1] for m in ms)
    out = np.zeros((k, n), np.float32)
    i = j = 0
    for m in ms:
        out[i : i + m.shape[0], j : j + m.shape[1]] = m
        i += m.shape[0]
        j += m.shape[1]
    return out


def _build_program():
    import concourse.bacc as bacc
    import concourse.tile as tile
    from concourse import mybir
    from concourse.masks import make_identity

    f32 = mybir.dt.float32
    AF = mybir.ActivationFunctionType
    ALU = mybir.AluOpType
    import os

    # float32r: single-pass fp32 matmul (full PE rate at N>=256) vs the
    # 2-pass full-precision fp32 path.  The BIR verifier requires every
    # producer of an fp32r-matmul operand to emit fp32r, so the weight slab
    # and all matmul-feeding tiles are allocated in that dtype directly.
    use_f32r = os.environ.get("KERNEL_MM_F32R", "1") == "1"
    mdt = mybir.dt.float32r if use_f32r else f32
    unr = (lambda ap: ap.bitcast(f32)) if use_f32r else (lambda ap: ap)

    nc = bacc.Bacc(
        "TRN2",
        target_bir_lowering=False,
        debug=False,
        enable_asserts=False,
        num_devices=N_CORES,
    )

    x_ap = nc.dram_tensor("x", (NPAD, F_IN), f32, kind="ExternalInput").ap()
    wslab_ap = nc.dram_tensor(
        "wslab", (128, WSLAB_COLS), mdt, kind="ExternalInput"
    ).ap()
    bslab_ap = nc.dram_tensor(
        "bslab", (128, BSLAB_COLS), f32, kind="ExternalInput"
    ).ap()
    y_ap = nc.dram_tensor("y", (NPAD,), f32, kind="ExternalOutput").ap()

    with tile.TileContext(nc) as tc:
        from contextlib import ExitStack

        with ExitStack() as ctx:
            const = ctx.enter_context(tc.tile_pool(name="const", bufs=1))
            bslab = const.tile([128, BSLAB_COLS], f32, name="bslab_sb")
            nc.scalar.dma_start(out=bslab[:], in_=bslab_ap[:])
            wslab = const.tile([128, WSLAB_COLS], mdt, name="wslab_sb")
            nc.scalar.dma_start(out=wslab[:], in_=wslab_ap[:])
            w = {
                name: wslab[0:rows, c0 : c0 + cols]
                for name, (c0, cols, rows) in _WSLAB.items()
            }
            w.update(
                {
                    name: bslab[0:rows, c0 : c0 + cols]
                    for name, (c0, cols, rows) in _BSLAB.items()
                }
            )
            ident = w["ident"]

            sb_in = ctx.enter_context(tc.tile_pool(name="sb_in", bufs=8))
            sb_xt = ctx.enter_context(tc.tile_pool(name="sb_xt", bufs=8))
            sb_g = ctx.enter_context(tc.tile_pool(name="sb_g", bufs=6))
            sb_mid = ctx.enter_context(tc.tile_pool(name="sb_mid", bufs=5))
            sb_y = ctx.enter_context(tc.tile_pool(name="sb_y", bufs=3))
            ps_xt = ctx.enter_context(
                tc.tile_pool(name="ps_xt", bufs=1, space="PSUM")
            )
            ps_g = ctx.enter_context(tc.tile_pool(name="ps_g", bufs=1, space="PSUM"))
            ps_m = ctx.enter_context(tc.tile_pool(name="ps_m", bufs=2, space="PSUM"))

            def build_group(n0, n, c):
                """Emit phase thunks for c consecutive chunks of n nodes.

                Returns {phase: [thunk, ...]}; the caller interleaves phases
                across groups so the Tile scheduler sees one wide pipeline
                instead of serial per-group chains.
                """
                nh = c // 2                 # XT halves
                nt = max(n // 128, 1)       # transpose subtiles per half
                p = min(n, 128)             # rows per subtile
                nj = c // 4
                rows2 = c * 16
                xts = [None] * nh
                h1qs = [None] * nj
                h2qs = [None] * nj
                g2sb = {}
                ysb_box = [None]

                def load_half(h):
                    base = n0 + h * 2 * n
                    xin = sb_in.tile([p, nt * 128], f32, tag="xin",
                                     name=f"xin_{n0}_{h}")
                    nc.sync.dma_start(
                        out=xin[:].rearrange("p (t k f) -> p t k f", t=nt, k=2, f=64),
                        in_=x_ap[base : base + 2 * n, :].rearrange(
                            "(t k p) f -> p t k f", t=nt, k=2, p=p
                        ),
                    )
                    xtp = ps_xt.tile([128, n], f32, tag="xtp",
                                     name=f"xtp_{n0}_{h}")
                    for t in range(nt):
                        nc.tensor.transpose(
                            xtp[:, t * p : (t + 1) * p],
                            xin[:, t * 128 : (t + 1) * 128],
                            ident[0:p, 0:p],
                        )
                    xt = sb_xt.tile([128, n], mdt, tag="xt", name=f"xt_{n0}_{h}")
                    nc.vector.tensor_copy(xt[:], xtp[:])
                    xts[h] = xt

                def g1_half(h):
                    if h1qs[h // 2] is None:
                        h1qs[h // 2] = sb_mid.tile(
                            [128, n], mdt, tag="h1q", name=f"h1q_{n0}_{h // 2}"
                        )
                    xt = xts[h]
                    pi = ps_g.tile([128, n], f32, tag="pg_i", name=f"pi_{n0}_{h}")
                    nc.tensor.matmul(pi[:], w["A_i"], xt[:], start=True, stop=True)
                    pc = ps_g.tile([128, n], f32, tag="pg_c", name=f"pc_{n0}_{h}")
                    nc.tensor.matmul(pc[:], w["A_c"], xt[:], start=True, stop=True)
                    ig = sb_g.tile([128, n], f32, tag="ig", name=f"ig_{n0}_{h}")
                    nc.scalar.activation(ig[:], pi[:], AF.Sigmoid, bias=w["b1_i"])
                    tg = sb_g.tile([128, n], f32, tag="tg", name=f"tg_{n0}_{h}")
                    nc.scalar.activation(tg[:], pc[:], AF.Tanh, bias=w["b1_c"])
                    cg = sb_g.tile([128, n], mdt, tag="cg", name=f"cg_{n0}_{h}")
                    nc.vector.tensor_mul(cg[:], ig[:], tg[:])
                    po = ps_g.tile([128, n], f32, tag="pg_o", name=f"po_{n0}_{h}")
                    nc.tensor.matmul(po[:], w["A_o"], xt[:], start=True, stop=False)
                    nc.tensor.matmul(po[:], w["Dwc1"], cg[:], start=False, stop=True)
                    og = sb_g.tile([128, n], f32, tag="og", name=f"og_{n0}_{h}")
                    nc.scalar.activation(og[:], po[:], AF.Sigmoid, bias=w["b1_o"])
                    crg = sb_g.tile([128, n], f32, tag="crg", name=f"crg_{n0}_{h}")
                    nc.gpsimd.tensor_scalar_max(crg[:], unr(cg[:]), 0.0)
                    thg = sb_g.tile([128, n], f32, tag="thg", name=f"thg_{n0}_{h}")
                    nc.scalar.activation(thg[:], crg[:], AF.Tanh)
                    hg = sb_g.tile([128, n], mdt, tag="hg", name=f"hg_{n0}_{h}")
                    nc.vector.tensor_mul(hg[:], og[:], thg[:])
                    p2 = ps_m.tile([64, n], f32, tag="pm", name=f"p2_{n0}_{h}")
                    nc.tensor.matmul(p2[:], w["Ac1"], hg[:], start=True, stop=True)
                    r0 = 64 * (h % 2)
                    nc.vector.tensor_scalar(
                        h1qs[h // 2][r0 : r0 + 64, :], p2[:], w["bc1"], 0.0,
                        op0=ALU.add, op1=ALU.max,
                    )

                def g2_ic(j):
                    if "i2" not in g2sb:
                        g2sb["i2"] = sb_g.tile([rows2, n], f32, tag="ig",
                                               name=f"i2_{n0}")
                        g2sb["t2"] = sb_g.tile([rows2, n], f32, tag="tg",
                                               name=f"t2_{n0}")
                    p3i = ps_g.tile([64, n], f32, tag="pg2_i", name=f"p3i_{n0}_{j}")
                    nc.tensor.matmul(p3i[:], w["G2_i"], h1qs[j][:], start=True, stop=True)
                    nc.scalar.activation(
                        g2sb["i2"][64 * j : 64 * j + 64, :], p3i[:], AF.Sigmoid,
                        bias=w["b2_i"][0:64],
                    )
                    p3c = ps_g.tile([64, n], f32, tag="pg2_c", name=f"p3c_{n0}_{j}")
                    nc.tensor.matmul(p3c[:], w["G2_c"], h1qs[j][:], start=True, stop=True)
                    nc.scalar.activation(
                        g2sb["t2"][64 * j : 64 * j + 64, :], p3c[:], AF.Tanh,
                        bias=w["b2_c"][0:64],
                    )

                def g2_tail():
                    i2, t2 = g2sb["i2"], g2sb["t2"]
                    c2t = sb_g.tile([rows2, n], mdt, tag="cg", name=f"c2t_{n0}")
                    nc.vector.tensor_mul(c2t[:], i2[:], t2[:])
                    o2 = sb_g.tile([rows2, n], f32, tag="og", name=f"o2_{n0}")
                    for j in range(nj):
                        p3o = ps_g.tile([64, n], f32, tag="pg_o",
                                        name=f"p3o_{n0}_{j}")
                        nc.tensor.matmul(p3o[:], w["G2_o"], h1qs[j][:],
                                         start=True, stop=False)
                        nc.tensor.matmul(
                            p3o[:], w["Dwc2"][64 * j : 64 * j + 64, :],
                            c2t[64 * j : 64 * j + 64, :],
                            start=False, stop=True,
                        )
                        nc.scalar.activation(
                            o2[64 * j : 64 * j + 64, :], p3o[:], AF.Sigmoid,
                            bias=w["b2_o"][0:64],
                        )
                    c2r = sb_g.tile([rows2, n], f32, tag="crg", name=f"c2r_{n0}")
                    nc.gpsimd.tensor_scalar_max(c2r[:], unr(c2t[:]), 0.0)
                    t2h = sb_g.tile([rows2, n], f32, tag="thg", name=f"t2h_{n0}")
                    nc.scalar.activation(t2h[:], c2r[:], AF.Tanh)
                    for j in range(nj):
                        h2qs[j] = sb_mid.tile([64, n], mdt, tag="h2q",
                                              name=f"h2q_{n0}_{j}")
                        nc.vector.tensor_mul(
                            h2qs[j][:],
                            o2[64 * j : 64 * j + 64, :],
                            t2h[64 * j : 64 * j + 64, :],
                        )

                def out_half(h):
                    if ysb_box[0] is None:
                        ysb_box[0] = sb_y.tile([2, nh * n], f32, tag="ysb",
                                               name=f"ysb_{n0}")
                    ysb = ysb_box[0]
                    hh = h % 2
                    p4 = ps_m.tile([128, n], f32, tag="pm", name=f"p4_{n0}_{h}")
                    nc.tensor.matmul(
                        p4[:],
                        w["Ac2"][32 * hh : 32 * (hh + 1), :],
                        h2qs[h // 2][32 * hh : 32 * hh + 32, :],
                        start=True, stop=True,
                    )
                    lr = sb_g.tile([128, n], mdt, tag="lr", name=f"lr_{n0}_{h}")
                    nc.vector.tensor_scalar(
                        lr[:], p4[:], w["bc2"], 0.0, op0=ALU.add, op1=ALU.max
                    )
                    p5 = ps_m.tile([2, n], f32, tag="pm", name=f"p5_{n0}_{h}")
                    nc.tensor.matmul(p5[:], w["Alin"], lr[:], start=True, stop=True)
                    nc.scalar.activation(
                        ysb[:, h * n : (h + 1) * n], p5[:], AF.Identity,
                        bias=w["blin"],
                    )
                    if h == nh - 1:
                        yv = y_ap[n0 : n0 + c * n].rearrange(
                            "(h t k p) -> k h t p", h=nh, t=nt, k=2
                        )
                        nc.gpsimd.dma_start(
                            out=yv,
                            in_=ysb[:].rearrange("k (h t p) -> k h t p", h=nh, t=nt),
                        )

                return {
                    "load": [lambda h=h: load_half(h) for h in range(nh)],
                    "g1": [lambda h=h: g1_half(h) for h in range(nh)],
                    "g2ic": [lambda j=j: g2_ic(j) for j in range(nj)],
                    "g2tail": [g2_tail],
                    "out": [lambda h=h: out_half(h) for h in range(nh)],
                }

            groups = [
                build_group(0, QN, 8),
                build_group(8 * QN, QN, 4),
                build_group(12 * QN, TAILN, 4),
            ]
            for phase in ("load", "g1", "g2ic", "g2tail", "out"):
                # round-robin across groups within each phase
                queues = [list(g[phase]) for g in groups]
                while any(queues):
                    for q in queues:
                        if q:
                            q.pop(0)()

    nc.compile()
    return nc


def _get_program():
    if "nc" not in _PROGRAM_CACHE:
        _PROGRAM_CACHE["nc"] = _build_program()
    return _PROGRAM_CACHE["nc"]


def _pack_weights(params):
    def a(v):
        return np.asarray(v, np.float32)

    g1, g2 = params["g1"], params["g2"]
    t2 = lambda v: np.tile(a(v), 2)
    t4 = lambda v: np.tile(a(v), 4)

    mats = {
        "A_i": _blkdiag(a(g1["W_i"]), a(g1["W_i"])),
        "A_c": _blkdiag(a(g1["W_c"]), a(g1["W_c"])),
        "A_o": _blkdiag(a(g1["W_o"]), a(g1["W_o"])),
        "Dwc1": np.diag(t2(g1["wc_o"])),
        "Ac1": _blkdiag(a(params["c1_w"]), a(params["c1_w"])),
        "G2_i": _blkdiag(*([a(g2["W_i"])] * 4)),
        "G2_c": _blkdiag(*([a(g2["W_c"])] * 4)),
        "G2_o": _blkdiag(*([a(g2["W_o"])] * 4)),
        "Dwc2": np.vstack([np.diag(t4(g2["wc_o"]))] * 2),
        "Ac2": np.vstack([_blkdiag(a(params["c2_w"]), a(params["c2_w"]))] * 4),
        "Alin": _blkdiag(a(params["lin_w"]), a(params["lin_w"])),
        "b1_i": t2(a(g1["cb_i"]) + a(g1["b_i"]))[:, None],
        "b1_c": t2(a(g1["cb_c"]) + a(g1["b_c"]))[:, None],
        "b1_o": t2(a(g1["cb_o"]) + a(g1["b_o"]))[:, None],
        "bc1": t2(params["c1_b"])[:, None],
        "b2_i": np.tile(a(g2["cb_i"]) + a(g2["b_i"]), 8)[:, None],
        "b2_c": np.tile(a(g2["cb_c"]) + a(g2["b_c"]), 8)[:, None],
        "b2_o": np.tile(a(g2["cb_o"]) + a(g2["b_o"]), 8)[:, None],
        "bc2": t2(params["c2_b"])[:, None],
        "blin": t2(params["lin_b"])[:, None],
        "ident": np.eye(128, dtype=np.float32),
    }
    wslab = np.zeros((128, WSLAB_COLS), np.float32)
    for name, (c0, cols, rows) in _WSLAB.items():
        m = np.asarray(mats[name], np.float32)
        assert m.shape == (rows, cols), (name, m.shape, (rows, cols))
        wslab[:rows, c0 : c0 + cols] = m
    bslab = np.zeros((128, BSLAB_COLS), np.float32)
    for name, (c0, cols, rows) in _BSLAB.items():
        m = np.asarray(mats[name], np.float32)
        assert m.shape == (rows, cols), (name, m.shape, (rows, cols))
        bslab[:rows, c0 : c0 + cols] = m
    return wslab, bslab


def kernel(params=None, x=None, edge_index=None, edge_weight=None, **_ignored):
    from concourse.bass_utils import run_bass_kernel_spmd

    x = np.asarray(x, np.float32)
    wslab, bslab = _pack_weights(params)

    in_maps = []
    for c in range(N_CORES):
        xs = np.zeros((NPAD, F_IN), np.float32)
        xs[:NLOC] = x[c * NLOC : (c + 1) * NLOC]
        in_maps.append({"x": xs, "wslab": wslab, "bslab": bslab})

    nc = _get_program()
    try:
        res = run_bass_kernel_spmd(nc, in_maps, list(range(N_CORES)))
    except ModuleNotFoundError:
        # BASS_TRACE set but the NTFF profile hook isn't available in this
        # environment; retry with tracing disabled.
        import os

        os.environ["BASS_NEVER_TRACE"] = "1"
        res = run_bass_kernel_spmd(nc, in_maps, list(range(N_CORES)))
    kernel._last_results = res
    y = np.concatenate([res.results[c]["y"][:NLOC] for c in range(N_CORES)])
    return np.concatenate([y, y]).reshape(2 * N_NODES, 1).astype(np.float32)


# revision 35
# speedup vs baseline: 1.0056x; 1.0056x over previous
"""Trainium2 Bass kernel for nn_CustomModel_4372276707887 (GCLSTM stack).

Mathematical structure: in the reference, every GCLSTM step runs with
H = C = 0, so each ChebConv acts on a zero matrix and contributes only its
bias; the forget gate multiplies C=0 and the second round of blocks is
discarded.  The whole model therefore reduces to a dense per-node chain

    I = sigmoid(x @ W_i + cb_i + b_i)
    T = tanh  (x @ W_c + cb_c + b_c)
    C = I * T
    O = sigmoid(x @ W_o + cb_o + wc_o * C + b_o)
    H = relu(O * tanh(C))                 (relu from the block)
    h = relu(H @ c1_w + c1_b)
    ... same gate block with g2 (32 -> 16) ...
    L = relu(h2 @ c2_w + c2_b)
    y = L @ lin_w + lin_b                 out = concat([y, y], axis=0)

which never touches edge_index / edge_weight.  The kernel shards the 50000
nodes across 8 NeuronCores (6250 each, padded to 6272) and evaluates the
chain in a transposed layout (features on partitions, nodes on the free
dim) with block-diagonal weight packing so every engine op runs with full
128-partition occupancy.  The wc_o * C terms are folded into the O-gate
matmul as an accumulating diagonal matmul on the PE.  All weights and
biases ship in a single [128, SLAB_COLS] slab (one DMA); node tiles load
with one strided DMA per half-quad and store with one DMA per quad.
"""

import sys

if "/opt/trn_rl_repo" not in sys.path:
    sys.path.insert(0, "/opt/trn_rl_repo")

import numpy as np

N_CORES = 8
N_NODES = 50000
F_IN = 64
NLOC = N_NODES // N_CORES      # 6250 nodes per core
QN = 512                       # nodes per chunk in a full group
TAILN = 32                     # chunk size of the tail mini-group (4 * 32 = 128)
NPAD = 12 * QN + 4 * TAILN     # 6272 padded nodes per core (octet + quad + tail)

# weight-slab column layout: name -> (col_offset, n_cols, n_rows)
_WSLAB = {
    "A_i": (0, 128, 128),
    "A_c": (128, 128, 128),
    "A_o": (256, 128, 128),
    "Dwc1": (384, 128, 128),
    "Ac1": (512, 64, 128),
    "G2_i": (576, 64, 128),
    "G2_c": (640, 64, 128),
    "G2_o": (704, 64, 128),
    "Dwc2": (768, 64, 128),
    "Ac2": (832, 128, 128),
    "Alin": (960, 2, 128),
}
WSLAB_COLS = 962
_BSLAB = {
    "ident": (9, 128, 128),
    "b1_i": (0, 1, 128),
    "b1_c": (1, 1, 128),
    "b1_o": (2, 1, 128),
    "bc1": (3, 1, 64),
    "b2_i": (4, 1, 128),
    "b2_c": (5, 1, 128),
    "b2_o": (6, 1, 128),
    "bc2": (7, 1, 128),
    "blin": (8, 1, 2),
}
BSLAB_COLS = 137

_PROGRAM_CACHE = {}


def _blkdiag(*ms):
    k = sum(m.shape[0] for m in ms)
    n = sum(m.shape[1] for m in ms)
    out = np.zeros((k, n), np.float32)
    i = j = 0
    for m in ms:
        out[i : i + m.shape[0], j : j + m.shape[1]] = m
        i += m.shape[0]
        j += m.shape[1]
    return out


def _build_program():
    import concourse.bacc as bacc
    import concourse.tile as tile
    from concourse import mybir
    from concourse.masks import make_identity

    f32 = mybir.dt.float32
    AF = mybir.ActivationFunctionType
    ALU = mybir.AluOpType
    import os

    # float32r: single-pass fp32 matmul (full PE rate at N>=256) vs the
    # 2-pass full-precision fp32 path.  The BIR verifier requires every
    # producer of an fp32r-matmul operand to emit fp32r, so the weight slab
    # and all matmul-feeding tiles are allocated in that dtype directly.
    use_f32r = os.environ.get("KERNEL_MM_F32R", "1") == "1"
    mdt = mybir.dt.float32r if use_f32r else f32
    unr = (lambda ap: ap.bitcast(f32)) if use_f32r else (lambda ap: ap)

    nc = bacc.Bacc(
        "TRN2",
        target_bir_lowering=False,
        debug=False,
        enable_asserts=False,
        num_devices=N_CORES,
    )

    x_ap = nc.dram_tensor("x", (NPAD, F_IN), f32, kind="ExternalInput").ap()
    wslab_ap = nc.dram_tensor(
        "wslab", (128, WSLAB_COLS), mdt, kind="ExternalInput"
    ).ap()
    bslab_ap = nc.dram_tensor(
        "bslab", (128, BSLAB_COLS), f32, kind="ExternalInput"
    ).ap()
    y_ap = nc.dram_tensor("y", (NPAD,), f32, kind="ExternalOutput").ap()

    with tile.TileContext(nc) as tc:
        from contextlib import ExitStack

        with ExitStack() as ctx:
            const = ctx.enter_context(tc.tile_pool(name="const", bufs=1))
            bslab = const.tile([128, BSLAB_COLS], f32, name="bslab_sb")
            nc.scalar.dma_start(out=bslab[:], in_=bslab_ap[:])
            wslab = const.tile([128, WSLAB_COLS], mdt, name="wslab_sb")
            nc.scalar.dma_start(out=wslab[:], in_=wslab_ap[:])
            w = {
                name: wslab[0:rows, c0 : c0 + cols]
                for name, (c0, cols, rows) in _WSLAB.items()
            }
            w.update(
                {
                    name: bslab[0:rows, c0 : c0 + cols]
                    for name, (c0, cols, rows) in _BSLAB.items()
                }
            )
            ident = w["ident"]

            sb_in = ctx.enter_context(tc.tile_pool(name="sb_in", bufs=8))
            sb_xt = ctx.enter_context(tc.tile_pool(name="sb_xt", bufs=8))
            sb_g = ctx.enter_context(tc.tile_pool(name="sb_g", bufs=6))
            sb_mid = ctx.enter_context(tc.tile_pool(name="sb_mid", bufs=5))
            sb_y = ctx.enter_context(tc.tile_pool(name="sb_y", bufs=3))
            ps_xt = ctx.enter_context(
                tc.tile_pool(name="ps_xt", bufs=1, space="PSUM")
            )
            ps_g = ctx.enter_context(tc.tile_pool(name="ps_g", bufs=1, space="PSUM"))
            ps_m = ctx.enter_context(tc.tile_pool(name="ps_m", bufs=2, space="PSUM"))

            def build_group(n0, n, c):
                """Emit phase thunks for c consecutive chunks of n nodes.

                Returns {phase: [thunk, ...]}; the caller interleaves phases
                across groups so the Tile scheduler sees one wide pipeline
                instead of serial per-group chains.
                """
                nh = c // 2                 # XT halves
                nt = max(n // 128, 1)       # transpose subtiles per half
                p = min(n, 128)             # rows per subtile
                nj = c // 4
                rows2 = c * 16
                xts = [None] * nh
                h1qs = [None] * nj
                h2qs = [None] * nj
                g2sb = {}
                ysb_box = [None]

                def load_half(h):
                    base = n0 + h * 2 * n
                    xin = sb_in.tile([p, nt * 128], f32, tag="xin",
                                     name=f"xin_{n0}_{h}")
                    # split wide loads so the first transposes start before
                    # the whole half has landed
                    nsplit = 2 if nt >= 4 else 1
                    ts_per = nt // nsplit
                    rows_per = 2 * n // nsplit
                    for q in range(nsplit):
                        nc.sync.dma_start(
                            out=xin[:, q * ts_per * 128 : (q + 1) * ts_per * 128]
                            .rearrange("p (t k f) -> p t k f", t=ts_per, k=2, f=64),
                            in_=x_ap[
                                base + q * rows_per : base + (q + 1) * rows_per, :
                            ].rearrange("(t k p) f -> p t k f", t=ts_per, k=2, p=p),
                        )
                    xtp = ps_xt.tile([128, n], f32, tag="xtp",
                                     name=f"xtp_{n0}_{h}")
                    for t in range(nt):
                        nc.tensor.transpose(
                            xtp[:, t * p : (t + 1) * p],
                            xin[:, t * 128 : (t + 1) * 128],
                            ident[0:p, 0:p],
                        )
                    xt = sb_xt.tile([128, n], mdt, tag="xt", name=f"xt_{n0}_{h}")
                    nc.vector.tensor_copy(xt[:], xtp[:])
                    xts[h] = xt

                def g1_half(h):
                    if h1qs[h // 2] is None:
                        h1qs[h // 2] = sb_mid.tile(
                            [128, n], mdt, tag="h1q", name=f"h1q_{n0}_{h // 2}"
                        )
                    xt = xts[h]
                    pi = ps_g.tile([128, n], f32, tag="pg_i", name=f"pi_{n0}_{h}")
                    nc.tensor.matmul(pi[:], w["A_i"], xt[:], start=True, stop=True)
                    pc = ps_g.tile([128, n], f32, tag="pg_c", name=f"pc_{n0}_{h}")
                    nc.tensor.matmul(pc[:], w["A_c"], xt[:], start=True, stop=True)
                    ig = sb_g.tile([128, n], f32, tag="ig", name=f"ig_{n0}_{h}")
                    nc.scalar.activation(ig[:], pi[:], AF.Sigmoid, bias=w["b1_i"])
                    tg = sb_g.tile([128, n], f32, tag="tg", name=f"tg_{n0}_{h}")
                    nc.scalar.activation(tg[:], pc[:], AF.Tanh, bias=w["b1_c"])
                    cg = sb_g.tile([128, n], mdt, tag="cg", name=f"cg_{n0}_{h}")
                    nc.vector.tensor_mul(cg[:], ig[:], tg[:])
                    po = ps_g.tile([128, n], f32, tag="pg_o", name=f"po_{n0}_{h}")
                    nc.tensor.matmul(po[:], w["A_o"], xt[:], start=True, stop=False)
                    nc.tensor.matmul(po[:], w["Dwc1"], cg[:], start=False, stop=True)
                    og = sb_g.tile([128, n], f32, tag="og", name=f"og_{n0}_{h}")
                    nc.scalar.activation(og[:], po[:], AF.Sigmoid, bias=w["b1_o"])
                    crg = sb_g.tile([128, n], f32, tag="crg", name=f"crg_{n0}_{h}")
                    nc.gpsimd.tensor_scalar_max(crg[:], unr(cg[:]), 0.0)
                    thg = sb_g.tile([128, n], f32, tag="thg", name=f"thg_{n0}_{h}")
                    nc.scalar.activation(thg[:], crg[:], AF.Tanh)
                    hg = sb_g.tile([128, n], mdt, tag="hg", name=f"hg_{n0}_{h}")
                    nc.vector.tensor_mul(hg[:], og[:], thg[:])
                    p2 = ps_m.tile([64, n], f32, tag="pm", name=f"p2_{n0}_{h}")
                    nc.tensor.matmul(p2[:], w["Ac1"], hg[:], start=True, stop=True)
                    r0 = 64 * (h % 2)
                    nc.vector.tensor_scalar(
                        h1qs[h // 2][r0 : r0 + 64, :], p2[:], w["bc1"], 0.0,
                        op0=ALU.add, op1=ALU.max,
                    )

                def g2_ic(j):
                    if "i2" not in g2sb:
                        g2sb["i2"] = sb_g.tile([rows2, n], f32, tag="ig",
                                               name=f"i2_{n0}")
                        g2sb["t2"] = sb_g.tile([rows2, n], f32, tag="tg",
                                               name=f"t2_{n0}")
                    p3i = ps_g.tile([64, n], f32, tag="pg2_i", name=f"p3i_{n0}_{j}")
                    nc.tensor.matmul(p3i[:], w["G2_i"], h1qs[j][:], start=True, stop=True)
                    nc.scalar.activation(
                        g2sb["i2"][64 * j : 64 * j + 64, :], p3i[:], AF.Sigmoid,
                        bias=w["b2_i"][0:64],
                    )
                    p3c = ps_g.tile([64, n], f32, tag="pg2_c", name=f"p3c_{n0}_{j}")
                    nc.tensor.matmul(p3c[:], w["G2_c"], h1qs[j][:], start=True, stop=True)
                    nc.scalar.activation(
                        g2sb["t2"][64 * j : 64 * j + 64, :], p3c[:], AF.Tanh,
                        bias=w["b2_c"][0:64],
                    )

                def g2_tail():
                    i2, t2 = g2sb["i2"], g2sb["t2"]
                    c2t = sb_g.tile([rows2, n], mdt, tag="cg", name=f"c2t_{n0}")
                    nc.vector.tensor_mul(c2t[:], i2[:], t2[:])
                    o2 = sb_g.tile([rows2, n], f32, tag="og", name=f"o2_{n0}")
                    for j in range(nj):
                        p3o = ps_g.tile([64, n], f32, tag="pg_o",
                                        name=f"p3o_{n0}_{j}")
                        nc.tensor.matmul(p3o[:], w["G2_o"], h1qs[j][:],
                                         start=True, stop=False)
                        nc.tensor.matmul(
                            p3o[:], w["Dwc2"][64 * j : 64 * j + 64, :],
                            c2t[64 * j : 64 * j + 64, :],
                            start=False, stop=True,
                        )
                        nc.scalar.activation(
                            o2[64 * j : 64 * j + 64, :], p3o[:], AF.Sigmoid,
                            bias=w["b2_o"][0:64],
                        )
                    c2r = sb_g.tile([rows2, n], f32, tag="crg", name=f"c2r_{n0}")
                    nc.gpsimd.tensor_scalar_max(c2r[:], unr(c2t[:]), 0.0)
                    t2h = sb_g.tile([rows2, n], f32, tag="thg", name=f"t2h_{n0}")
                    nc.scalar.activation(t2h[:], c2r[:], AF.Tanh)
                    for j in range(nj):
                        h2qs[j] = sb_mid.tile([64, n], mdt, tag="h2q",
                                              name=f"h2q_{n0}_{j}")
                        nc.vector.tensor_mul(
                            h2qs[j][:],
                            o2[64 * j : 64 * j + 64, :],
                            t2h[64 * j : 64 * j + 64, :],
                        )

                def out_half(h):
                    if ysb_box[0] is None:
                        ysb_box[0] = sb_y.tile([2, nh * n], f32, tag="ysb",
                                               name=f"ysb_{n0}")
                    ysb = ysb_box[0]
                    hh = h % 2
                    p4 = ps_m.tile([128, n], f32, tag="pm", name=f"p4_{n0}_{h}")
                    nc.tensor.matmul(
                        p4[:],
                        w["Ac2"][32 * hh : 32 * (hh + 1), :],
                        h2qs[h // 2][32 * hh : 32 * hh + 32, :],
                        start=True, stop=True,
                    )
                    lr = sb_g.tile([128, n], mdt, tag="lr", name=f"lr_{n0}_{h}")
                    nc.vector.tensor_scalar(
                        lr[:], p4[:], w["bc2"], 0.0, op0=ALU.add, op1=ALU.max
                    )
                    p5 = ps_m.tile([2, n], f32, tag="pm", name=f"p5_{n0}_{h}")
                    nc.tensor.matmul(p5[:], w["Alin"], lr[:], start=True, stop=True)
                    nc.scalar.activation(
                        ysb[:, h * n : (h + 1) * n], p5[:], AF.Identity,
                        bias=w["blin"],
                    )
                    if h == nh - 1:
                        yv = y_ap[n0 : n0 + c * n].rearrange(
                            "(h t k p) -> k h t p", h=nh, t=nt, k=2
                        )
                        nc.gpsimd.dma_start(
                            out=yv,
                            in_=ysb[:].rearrange("k (h t p) -> k h t p", h=nh, t=nt),
                        )

                return {
                    "load": [lambda h=h: load_half(h) for h in range(nh)],
                    "g1": [lambda h=h: g1_half(h) for h in range(nh)],
                    "g2ic": [lambda j=j: g2_ic(j) for j in range(nj)],
                    "g2tail": [g2_tail],
                    "out": [lambda h=h: out_half(h) for h in range(nh)],
                }

            groups = [
                build_group(0, QN, 8),
                build_group(8 * QN, QN, 4),
                build_group(12 * QN, TAILN, 4),
            ]
            for phase in ("load", "g1", "g2ic", "g2tail", "out"):
                # round-robin across groups within each phase
                queues = [list(g[phase]) for g in groups]
                while any(queues):
                    for q in queues:
                        if q:
                            q.pop(0)()

    nc.compile()
    return nc


def _get_program():
    if "nc" not in _PROGRAM_CACHE:
        _PROGRAM_CACHE["nc"] = _build_program()
    return _PROGRAM_CACHE["nc"]


def _pack_weights(params):
    def a(v):
        return np.asarray(v, np.float32)

    g1, g2 = params["g1"], params["g2"]
    t2 = lambda v: np.tile(a(v), 2)
    t4 = lambda v: np.tile(a(v), 4)

    mats = {
        "A_i": _blkdiag(a(g1["W_i"]), a(g1["W_i"])),
        "A_c": _blkdiag(a(g1["W_c"]), a(g1["W_c"])),
        "A_o": _blkdiag(a(g1["W_o"]), a(g1["W_o"])),
        "Dwc1": np.diag(t2(g1["wc_o"])),
        "Ac1": _blkdiag(a(params["c1_w"]), a(params["c1_w"])),
        "G2_i": _blkdiag(*([a(g2["W_i"])] * 4)),
        "G2_c": _blkdiag(*([a(g2["W_c"])] * 4)),
        "G2_o": _blkdiag(*([a(g2["W_o"])] * 4)),
        "Dwc2": np.vstack([np.diag(t4(g2["wc_o"]))] * 2),
        "Ac2": np.vstack([_blkdiag(a(params["c2_w"]), a(params["c2_w"]))] * 4),
        "Alin": _blkdiag(a(params["lin_w"]), a(params["lin_w"])),
        "b1_i": t2(a(g1["cb_i"]) + a(g1["b_i"]))[:, None],
        "b1_c": t2(a(g1["cb_c"]) + a(g1["b_c"]))[:, None],
        "b1_o": t2(a(g1["cb_o"]) + a(g1["b_o"]))[:, None],
        "bc1": t2(params["c1_b"])[:, None],
        "b2_i": np.tile(a(g2["cb_i"]) + a(g2["b_i"]), 8)[:, None],
        "b2_c": np.tile(a(g2["cb_c"]) + a(g2["b_c"]), 8)[:, None],
        "b2_o": np.tile(a(g2["cb_o"]) + a(g2["b_o"]), 8)[:, None],
        "bc2": t2(params["c2_b"])[:, None],
        "blin": t2(params["lin_b"])[:, None],
        "ident": np.eye(128, dtype=np.float32),
    }
    wslab = np.zeros((128, WSLAB_COLS), np.float32)
    for name, (c0, cols, rows) in _WSLAB.items():
        m = np.asarray(mats[name], np.float32)
        assert m.shape == (rows, cols), (name, m.shape, (rows, cols))
        wslab[:rows, c0 : c0 + cols] = m
    bslab = np.zeros((128, BSLAB_COLS), np.float32)
    for name, (c0, cols, rows) in _BSLAB.items():
        m = np.asarray(mats[name], np.float32)
        assert m.shape == (rows, cols), (name, m.shape, (rows, cols))
        bslab[:rows, c0 : c0 + cols] = m
    return wslab, bslab


def kernel(params=None, x=None, edge_index=None, edge_weight=None, **_ignored):
    from concourse.bass_utils import run_bass_kernel_spmd

    x = np.asarray(x, np.float32)
    wslab, bslab = _pack_weights(params)

    in_maps = []
    for c in range(N_CORES):
        xs = np.zeros((NPAD, F_IN), np.float32)
        xs[:NLOC] = x[c * NLOC : (c + 1) * NLOC]
        in_maps.append({"x": xs, "wslab": wslab, "bslab": bslab})

    nc = _get_program()
    try:
        res = run_bass_kernel_spmd(nc, in_maps, list(range(N_CORES)))
    except ModuleNotFoundError:
        # BASS_TRACE set but the NTFF profile hook isn't available in this
        # environment; retry with tracing disabled.
        import os

        os.environ["BASS_NEVER_TRACE"] = "1"
        res = run_bass_kernel_spmd(nc, in_maps, list(range(N_CORES)))
    kernel._last_results = res
    y = np.concatenate([res.results[c]["y"][:NLOC] for c in range(N_CORES)])
    return np.concatenate([y, y]).reshape(2 * N_NODES, 1).astype(np.float32)
